# revision 1
# baseline (speedup 1.0000x reference)
"""Neighbourhood attention block (7x7 window) on 8 Trainium2 NeuronCores.

Full inputs -> full output. Sharding: core = b*4 + g owns batch b and query
rows 16g..16g+15 (all 6 heads). Each core gets a 24-row halo slice of x,
transposed to feature-major and laid out in column-major token order
(token = wc*24 + r, wc = padded column 0..71, r = local row 0..23) so that
every 16-col x 24-row key window is a contiguous 384-token run: key chunks
(128 keys) and query blocks (128 queries = 8 cols x 16 rows) are then plain
2D slices, as required for matmul stationary operands.

Softmax runs without max-subtraction (scores are O(1)): keys on partitions,
probs = exp(s/8) * mask01 (bf16); the denominator comes free from a
ones-column appended to V (PV output col 64 of each 65-col head slot);
reciprocal + normalization are per-partition ops on the token-major PV
output; attn is then PE-transposed to feature-major for the out-projection.
"""
import sys

sys.path.insert(0, "/opt/trn_rl_repo")

import numpy as np
import ml_dtypes

import concourse.bass as bass
import concourse.mybir as mybir
from concourse import bacc
from concourse.tile import TileContext
from concourse.bass_utils import run_bass_kernel_spmd
from concourse.bass import broadcast_tensor_aps

F32 = mybir.dt.float32
BF16 = mybir.dt.bfloat16
AF = mybir.ActivationFunctionType

D = 384
NH = 6
E = 64
NCORES = 8
TOK = 1728          # 72 padded cols x 24 rows, column-major
SCALE = 0.125       # 1/sqrt(64)


def emit(nc):
    xT = nc.dram_tensor("xT", [D, TOK], F32, kind="ExternalInput").ap()
    wqkvT = nc.dram_tensor("wqkvT", [D, 3 * D], F32, kind="ExternalInput").ap()
    woutT = nc.dram_tensor("woutT", [D, D], F32, kind="ExternalInput").ap()
    ident = nc.dram_tensor("ident", [128, 128], F32, kind="ExternalInput").ap()
    mask = nc.dram_tensor("mask", [128, 8 * 3 * 128], BF16, kind="ExternalInput").ap()
    out = nc.dram_tensor("out", [8, 128, D], F32, kind="ExternalOutput").ap()

    with TileContext(nc) as tc:
        with tc.tile_pool(name="persist", bufs=1) as pp:
            xT_sb = [pp.tile([128, TOK], F32, tag=f"xT{i}", name=f"xT{i}")
                     for i in range(3)]
            w1_sb = [pp.tile([128, 3 * D], F32, tag=f"w1{i}", name=f"w1{i}")
                     for i in range(3)]
            w2_sb = [pp.tile([128, D], F32, tag=f"w2{i}", name=f"w2{i}")
                     for i in range(3)]
            id_sb = pp.tile([128, 128], F32, tag="id", name="idsb")
            mk_sb = pp.tile([128, 8 * 3 * 128], BF16, tag="mk", name="mksb")
            qT_sb = pp.tile([128, 3 * 1024], F32, tag="qT", name="qTsb")
            kT_sb = [pp.tile([128, TOK], F32, tag=f"kT{i}", name=f"kT{i}")
                     for i in range(3)]
            v_sb = pp.tile([128, 24 * 390], BF16, tag="v", name="vsb")

            for i in range(3):
                nc.sync.dma_start(out=xT_sb[i][:], in_=xT[i * 128:(i + 1) * 128, :])
                nc.sync.dma_start(out=w1_sb[i][:], in_=wqkvT[i * 128:(i + 1) * 128, :])
                nc.sync.dma_start(out=w2_sb[i][:], in_=woutT[i * 128:(i + 1) * 128, :])
            nc.sync.dma_start(out=id_sb[:], in_=ident[:])
            nc.sync.dma_start(out=mk_sb[:], in_=mask[:])

            # ones-columns of v (col 64 of each 65-col head slot)
            vv = v_sb[:].rearrange("p (c h e) -> p c h e", h=NH, e=65)
            nc.gpsimd.memset(vv[:, :, :, 64:65], 1.0)

            # ---- qkv projections ----
            with tc.tile_pool(name="qkps", bufs=3, space="PSUM") as qkp:
                # q^T: owned tokens (cols 4..67, rows 3..18), col-major
                for f in range(3):
                    for t in range(2):
                        ps = qkp.tile([128, 512], F32, tag="qk", name="qkps")
                        for d in range(3):
                            xv = xT_sb[d][:].rearrange("p (w r) -> p w r", r=24)
                            nc.tensor.matmul(
                                ps[:],
                                lhsT=w1_sb[d][:, f * 128:(f + 1) * 128],
                                rhs=xv[:, 4 + 32 * t:4 + 32 * (t + 1), 3:19],
                                start=(d == 0), stop=(d == 2),
                            )
                        nc.vector.tensor_copy(
                            out=qT_sb[:, f * 1024 + t * 512:f * 1024 + (t + 1) * 512],
                            in_=ps[:])
                # k^T over all 1728 tokens (zero pads give k=0)
                for f in range(3):
                    for t in range(4):
                        w = 512 if t < 3 else 192
                        ps = qkp.tile([128, 512], F32, tag="qk", name="qkps")
                        for d in range(3):
                            nc.tensor.matmul(
                                ps[:, :w],
                                lhsT=w1_sb[d][:, 384 + f * 128:384 + (f + 1) * 128],
                                rhs=xT_sb[d][:, t * 512:t * 512 + w],
                                start=(d == 0), stop=(d == 2),
                            )
                        nc.vector.tensor_copy(
                            out=kT_sb[f][:, t * 512:t * 512 + w], in_=ps[:, :w])
                # v in key-chunk layout [128 keys, 6*65] per (bw, c)
                for ch in range(24):
                    bw, c = divmod(ch, 3)
                    k0 = 192 * bw + 128 * c
                    ps = qkp.tile([128, 384], F32, tag="vps", name="vps")
                    for d in range(3):
                        nc.tensor.matmul(
                            ps[:],
                            lhsT=xT_sb[d][:, k0:k0 + 128],
                            rhs=w1_sb[d][:, 768:1152],
                            start=(d == 0), stop=(d == 2),
                        )
                    nc.vector.tensor_copy(
                        out=vv[:, ch, :, 0:64],
                        in_=ps[:].rearrange("p (h e) -> p h e", e=64))

            # ---- attention + output projection ----
            with tc.tile_pool(name="spool", bufs=2, space="PSUM") as spool, \
                 tc.tile_pool(name="pvpool", bufs=1, space="PSUM") as pvpool, \
                 tc.tile_pool(name="trpool", bufs=1, space="PSUM") as trpool, \
                 tc.tile_pool(name="fpool", bufs=1, space="PSUM") as fpool, \
                 tc.tile_pool(name="work", bufs=6) as wp, \
                 tc.tile_pool(name="work2", bufs=2) as wp2:
                for bw in range(8):
                    e_tiles = []
                    for c in range(3):
                        e_sb = wp.tile([128, 768], BF16, tag="e", name="esb")
                        k0 = 192 * bw + 128 * c
                        m1 = mk_sb[:, (bw * 3 + c) * 128:(bw * 3 + c + 1) * 128]
                        m3 = m1.rearrange("p (o q) -> p o q", o=1)
                        for ph in range(3):
                            # pair tile: head-even -> bank 0 (cols 0:128),
                            # head-odd -> bank 1 (cols 512:640); one matmul
                            # group per bank (HW requirement)
                            sps = spool.tile([128, 1024], F32, tag="s",
                                             name="sps")
                            for par in range(2):
                                h = 2 * ph + par
                                nc.tensor.matmul(
                                    sps[:, par * 512:par * 512 + 128],
                                    lhsT=kT_sb[ph][par * 64:par * 64 + 64,
                                                   k0:k0 + 128],
                                    rhs=qT_sb[par * 64:par * 64 + 64,
                                              ph * 1024 + bw * 128:
                                              ph * 1024 + (bw + 1) * 128],
                                    start=True, stop=True,
                                    tile_position=(par * 64, 0),
                                )
                            sps3 = sps[:].rearrange(
                                "p (b q) -> p b q", q=512)[:, :, 0:128]
                            e3 = e_sb[:, ph * 256:(ph + 1) * 256].rearrange(
                                "p (b q) -> p b q", q=128)
                            nc.scalar.activation(out=e3, in_=sps3, func=AF.Exp,
                                                 scale=SCALE)
                            a, b = broadcast_tensor_aps(e3, m3)
                            nc.vector.tensor_mul(out=e3, in0=a, in1=b)
                        e_tiles.append(e_sb)
                    at = wp.tile([128, 384], F32, tag="at", name="atsb")
                    aTt = wp.tile([128, 384], F32, tag="aTt", name="aTt")
                    for ph in range(3):
                        pv = pvpool.tile([128, 1024], F32, tag="pv", name="pvps")
                        rc = wp.tile([128, 2], F32, tag="rc", name="rcsb")
                        for par in range(2):
                            h = 2 * ph + par
                            for c in range(3):
                                nc.tensor.matmul(
                                    pv[:, par * 512:par * 512 + 65],
                                    lhsT=e_tiles[c][:, (2 * ph) * 128 + par * 128:
                                                    (2 * ph) * 128 + (par + 1) * 128],
                                    rhs=v_sb[:, (bw * 3 + c) * 390 + h * 65:
                                             (bw * 3 + c) * 390 + (h + 1) * 65],
                                    start=(c == 0), stop=(c == 2),
                                )
                            nc.vector.reciprocal_approx_fast(
                                out=rc[:, par:par + 1],
                                in_=pv[:, par * 512 + 64:par * 512 + 65])
                        pv3 = pv[:].rearrange("p (b q) -> p b q", q=512)[:, :, 0:64]
                        rc3 = rc[:].rearrange("p (h o) -> p h o", o=1)
                        at3 = at[:, ph * 128:(ph + 1) * 128].rearrange(
                            "p (h e) -> p h e", e=64)
                        a, b = broadcast_tensor_aps(pv3, rc3)
                        nc.vector.tensor_mul(out=at3, in0=a, in1=b)
                    # transpose attn [128 q, 384 f] -> attnT tiles [128 f, 128 q]
                    for d3 in range(3):
                        trp = trpool.tile([128, 128], F32, tag="tr", name="trps")
                        nc.tensor.transpose(
                            out=trp[:], in_=at[:, d3 * 128:(d3 + 1) * 128],
                            identity=id_sb[:])
                        nc.scalar.copy(
                            out=aTt[:, d3 * 128:(d3 + 1) * 128], in_=trp[:])
                    fps = fpool.tile([128, 384], F32, tag="f", name="fps")
                    for d3 in range(3):
                        nc.tensor.matmul(
                            fps[:],
                            lhsT=aTt[:, d3 * 128:(d3 + 1) * 128],
                            rhs=w2_sb[d3][:],
                            start=(d3 == 0), stop=(d3 == 2),
                        )
                    ob = wp2.tile([128, 384], F32, tag="ob", name="obsb")
                    nc.scalar.copy(out=ob[:], in_=fps[:])
                    nc.sync.dma_start(out=out[bw], in_=ob[:])
    return nc


def full_neighbourhood_mask():
    """[4096, 4096] bool, True where key inside query's 7x7 clipped window."""
    hp = np.arange(64)
    sh = np.clip(hp - 3, 0, 57)
    hr = np.arange(64)
    rowv = (hr[None, :] >= sh[:, None]) & (hr[None, :] < (sh + 7)[:, None])
    m = rowv[:, None, :, None] & rowv[None, :, None, :]  # [qh, qw, kh, kw]
    return m.reshape(64 * 64, 64 * 64)


def core_mask_arr(g, fullmask):
    """bf16 [128, 8*3*128]: keys-on-partitions masks for row-group g.

    key index: window pos p = 128*c + ki, p = wl*24 + r (wl = key col
    - (8*bw - 4), r = local row); query index qi = qc*16 + qr.
    """
    out = np.zeros((8, 3, 128, 128), np.float32)
    qr = np.arange(16)
    qc = np.arange(8)
    for bw in range(8):
        p = np.arange(384)
        wl, r = p // 24, p % 24
        krow = 16 * g - 3 + r
        kcol = 8 * bw - 4 + wl
        kvalid = (krow >= 0) & (krow < 64) & (kcol >= 0) & (kcol < 64)
        ktok = np.clip(krow, 0, 63) * 64 + np.clip(kcol, 0, 63)
        qrow = 16 * g + qr
        qcol = 8 * bw + qc
        # qi = qc*16 + qr -> qc outer, qr inner
        qtok = (qrow[None, :] * 64 + qcol[:, None]).ravel()
        m = fullmask[qtok[None, :], ktok[:, None].astype(np.intp)]  # [384, 128]
        m = m & kvalid[:, None]
        out[bw] = m.reshape(3, 128, 128)
    return np.ascontiguousarray(
        out.transpose(2, 0, 1, 3).reshape(128, 8 * 3 * 128)
    ).astype(ml_dtypes.bfloat16)


_NC_CACHE = {}


def build():
    if "nc" not in _NC_CACHE:
        nc = bacc.Bacc("TRN2", target_bir_lowering=False, debug=False)
        emit(nc)
        nc.compile()
        _NC_CACHE["nc"] = nc
    return _NC_CACHE["nc"]


def make_in_maps(x, w_qkv, w_out):
    x = np.asarray(x, np.float32)
    wqkvT = np.ascontiguousarray(np.asarray(w_qkv, np.float32).T)
    woutT = np.ascontiguousarray(np.asarray(w_out, np.float32).T)
    ident = np.eye(128, dtype=np.float32)
    fullmask = full_neighbourhood_mask()
    gmasks = [core_mask_arr(g, fullmask) for g in range(4)]
    in_maps = []
    for core in range(NCORES):
        b, g = core // 4, core % 4
        rows = np.arange(16 * g - 3, 16 * g + 21)
        xs = np.zeros((24, 72, D), np.float32)  # [r, wc, D]
        valid = (rows >= 0) & (rows < 64)
        xs[valid, 4:68] = x[b, rows[valid]]
        # col-major tokens: token = wc*24 + r
        xT = np.ascontiguousarray(xs.transpose(2, 1, 0).reshape(D, 72 * 24))
        in_maps.append({
            "xT": xT, "wqkvT": wqkvT, "woutT": woutT,
            "ident": ident, "mask": gmasks[g],
        })
    return in_maps


def gather(results):
    full = np.zeros((2, 64, 64, D), np.float32)
    for core in range(NCORES):
        b, g = core // 4, core % 4
        o = results[core]["out"]  # [bw, qi = qc*16 + qr, f]
        o = o.reshape(8, 8, 16, D).transpose(2, 0, 1, 3).reshape(16, 64, D)
        full[b, 16 * g:16 * g + 16] = o
    return full


def kernel(x, w_qkv, w_out):
    nc = build()
    in_maps = make_in_maps(x, w_qkv, w_out)
    res = run_bass_kernel_spmd(nc, in_maps, core_ids=list(range(NCORES)))
    return gather(res.results)


def np_reference(x, w_qkv, w_out):
    """Plain-numpy port of reference.py for offline validation."""
    B, H, W, Dd = x.shape
    nh = Dd // E
    N = H * W
    qkv = x.reshape(B * N, Dd) @ w_qkv.T
    qkv = qkv.reshape(B, N, 3, nh, E).transpose(2, 0, 3, 1, 4)
    q, k, v = qkv[0], qkv[1], qkv[2]
    m = full_neighbourhood_mask()
    s = np.einsum("bnqe,bnke->bnqk", q, k) * (1.0 / np.sqrt(E))
    s = np.where(m[None, None], s, -np.inf)
    s = s - s.max(-1, keepdims=True)
    p = np.exp(s)
    p /= p.sum(-1, keepdims=True)
    o = np.einsum("bnqk,bnke->bnqe", p, v)
    o = o.transpose(0, 2, 1, 3).reshape(B, H, W, Dd)
    return o @ w_out.T


if __name__ == "__main__":
    from concourse.bass_interp import CoreSim
    rng = np.random.default_rng(0)
    x = rng.standard_normal((2, 64, 64, D), dtype=np.float32)
    w_qkv = (rng.standard_normal((3 * D, D)) * 0.02).astype(np.float32)
    w_out = (rng.standard_normal((D, D)) * 0.02).astype(np.float32)
    expected = np_reference(x, w_qkv, w_out)
    nc = build()
    in_maps = make_in_maps(x, w_qkv, w_out)
    core = int(sys.argv[1]) if len(sys.argv) > 1 else 0
    sim = CoreSim(nc)
    for kk, v in in_maps[core].items():
        sim.tensor(kk)[:] = v
    sim.simulate()
    got = np.array(sim.tensor("out"))
    b, g = core // 4, core % 4
    got = got.reshape(8, 8, 16, D).transpose(2, 0, 1, 3).reshape(16, 64, D)
    exp = expected[b, 16 * g:16 * g + 16]
    rel = np.linalg.norm(got - exp) / np.linalg.norm(exp)
    print(f"core {core}: rel_l2={rel:.3e} "
          f"absmax_rel={np.abs(got - exp).max() / np.abs(exp).max():.3e}")

